# revision 3
# baseline (speedup 1.0000x reference)
"""Bloom-style attention block (QKV proj + ALiBi causal attention + dense) on 8
Trainium2 NeuronCores, tensor-parallel over heads (4 heads per core), partial
dense outputs all-reduced on the host.

v2 layout strategy (all-bf16 data path, f32 PSUM accumulation):
  - xT      [128, 32, 4096] bf16 : x^T tiled   (h = 128*ht + p, t free)
  - q,k,v stay RESIDENT in SBUF between projection and attention (no DRAM
    scratch roundtrip):  qk_sb [128, 8, 4096] bf16, v_sb [128, 32, 512] bf16.
  - scores are computed transposed  sT[k, q]  so softmax sums over the
    partition axis reduce via a ones-vector matmul, p^T feeds attn@v
    directly, and the ALiBi bias+causal mask fold into one precomputed
    multiplicative tile  B = exp(slope*(j-i)) * (j<=i).
  - stage-1 matmuls use N=512 moving dim so bf16 FWL weight loads stay
    hidden under the rhs stream.
"""

import sys

sys.path.insert(0, "/opt/trn_rl_repo")

import math

import ml_dtypes
import numpy as np

B, S, H, NH = 2, 2048, 4096, 32
HD = H // NH          # 128
N_CORES = 8
HPC = NH // N_CORES   # 4 heads per core
T = B * S             # 4096 tokens
SCALE = HD ** -0.5

F32 = np.float32
BF16 = ml_dtypes.bfloat16


def _alibi_slopes(n: int) -> np.ndarray:
    cp2 = 2 ** math.floor(math.log2(n))
    base = 2.0 ** (-(2.0 ** (-(math.log2(cp2) - 3))))
    slopes = base ** np.arange(1, cp2 + 1, dtype=np.float64)
    if cp2 != n:
        extra_base = 2.0 ** (-(2.0 ** (-(math.log2(2 * cp2) - 3))))
        rem = min(cp2, n - cp2)
        extra = extra_base ** np.arange(1, 1 + 2 * rem, 2, dtype=np.float64)
        slopes = np.concatenate([slopes, extra])
    return slopes.astype(np.float64)


def heads_for_core(c: int) -> list[int]:
    # Interleaved so each head-slot j holds heads {8j..8j+7} across cores:
    # keeps the SPMD program uniform while per-slot ALiBi tile skipping uses
    # the slot's weakest decay (head 8j+7).
    return [8 * j + c for j in range(HPC)]


_SLOPES = _alibi_slopes(NH)

# A k-tile is skipped when every B entry satisfies slope*rel < THRESH: its
# softmax terms are < e^THRESH relative to the (>= e^0-ish) diagonal term,
# far below f32 accumulation resolution.
_THRESH = -25.0


def _keep_k_tiles(j: int, q0: int) -> list[int]:
    min_slope = _SLOPES[8 * j + 7]
    keep = []
    for kt in range(4 * q0 + 4):
        max_rel = 128 * kt + 127 - 512 * q0  # max over tile of (k - q)
        if max_rel >= 0 or min_slope * max_rel > _THRESH:
            keep.append(kt)
    return keep


def _build_program(trips: int = 1, stages: str = "123"):
    from concourse import bacc
    import concourse.tile as tile
    import concourse.mybir as mybir

    f32 = mybir.dt.float32
    f32r = mybir.dt.float32r
    bf16 = mybir.dt.bfloat16
    AF = mybir.ActivationFunctionType
    MULT = mybir.AluOpType.mult

    nc = bacc.Bacc("TRN2", target_bir_lowering=False, debug=False)

    xT = nc.dram_tensor("xT", [128, 32, T], bf16, kind="ExternalInput")
    wqkT = nc.dram_tensor("wqkT", [128, 32, 1024], bf16, kind="ExternalInput")
    wvT = nc.dram_tensor("wvT", [128, 32, 512], bf16, kind="ExternalInput")
    bqk = nc.dram_tensor("bqk", [128, 8], f32, kind="ExternalInput")
    bv = nc.dram_tensor("bv", [1, 512], bf16, kind="ExternalInput")
    btil = nc.dram_tensor("btil", [HPC, 128, 16, 512], bf16, kind="ExternalInput")
    ones_rb = nc.dram_tensor("ones_rb", [1, 128], bf16, kind="ExternalInput")
    ones_sq = nc.dram_tensor("ones_sq", [128, 128], bf16, kind="ExternalInput")
    wdT = nc.dram_tensor("wdT", [128, HPC, H], bf16, kind="ExternalInput")
    out = nc.dram_tensor("out", [T, H], bf16, kind="ExternalOutput")

    TBLK = 512
    NB = T // TBLK  # 8 token blocks

    with tile.TileContext(nc) as tc:
        with tc.tile_pool(name="const", bufs=1) as pconst, \
             tc.tile_pool(name="persist", bufs=1) as ppersist:
            ones_sq_sb = pconst.tile([128, 128], bf16)
            nc.sync.dma_start(out=ones_sq_sb[:, :], in_=ones_sq[:, :])
            ones_rowb = pconst.tile([1, 128], bf16)
            nc.sync.dma_start(out=ones_rowb[:, :], in_=ones_rb[:, :])
            bqk_sb = pconst.tile([128, 8], f32)
            nc.sync.dma_start(out=bqk_sb[:, :], in_=bqk[:, :])
            bv_sb = pconst.tile([1, 512], bf16)
            nc.sync.dma_start(out=bv_sb[:, :], in_=bv[:, :])

            # SBUF-resident q,k (lifetime: whole program)
            qk_sb = ppersist.tile([128, 8, T], bf16)

            def _stages():
                # Pool nesting is LIFO; v's scope encloses attention+dense
                # (32 KB idle during dense is affordable), attnout nests
                # inside v, per-stage pools innermost.

                # ---------------- stage 1a: q,k projection ----------------
                with tc.tile_pool(name="w1a", bufs=1) as pw, \
                     tc.tile_pool(name="x1a", bufs=2) as px, \
                     tc.tile_pool(name="ps1", bufs=4, space="PSUM") as pps:
                    w_sb = pw.tile([128, 32, 1024], bf16)
                    for wc in range(4):
                        nc.scalar.dma_start(
                            out=w_sb[:, 8 * wc:8 * (wc + 1), :],
                            in_=wqkT[:, 8 * wc:8 * (wc + 1), :])
                    for tb in range(NB):
                        x_sb = px.tile([128, 32, TBLK], bf16, tag="x")
                        for xc in range(2):
                            nc.scalar.dma_start(
                                out=x_sb[:, 16 * xc:16 * (xc + 1), :],
                                in_=xT[:, 16 * xc:16 * (xc + 1),
                                       TBLK * tb:TBLK * (tb + 1)],
                            )
                        for ji in range(8):  # 4 q slots then 4 k slots
                            ps = pps.tile([128, TBLK], f32, tag="ps1")
                            for h in range(32):
                                nc.tensor.matmul(
                                    ps[:, :],
                                    w_sb[:, h, 128 * ji:128 * (ji + 1)],
                                    x_sb[:, h, :],
                                    start=(h == 0), stop=(h == 31),
                                )
                            nc.scalar.activation(
                                qk_sb[:, ji, TBLK * tb:TBLK * (tb + 1)],
                                ps[:, :], AF.Identity,
                                bias=bqk_sb[:, ji:ji + 1],
                            )

                with tc.tile_pool(name="ao", bufs=1) as pao, \
                     tc.tile_pool(name="v2", bufs=1) as pv:
                    attnoutT = pao.tile([128, HPC, T], bf16)
                    v_sb = pv.tile([128, 32, 512], bf16)
                    # ---------------- stage 1b: v projection ----------------
                    with tc.tile_pool(name="x1b", bufs=2) as px, \
                         tc.tile_pool(name="w1b", bufs=1) as pw, \
                         tc.tile_pool(name="ps1b", bufs=4, space="PSUM") as pps:
                        wv_sb = pw.tile([128, 32, 512], bf16)
                        for wc in range(4):
                            nc.scalar.dma_start(
                                out=wv_sb[:, 8 * wc:8 * (wc + 1), :],
                                in_=wvT[:, 8 * wc:8 * (wc + 1), :])
                        for tb in range(2 * NB):
                            x_sb = px.tile([128, 32, 256], bf16, tag="x")
                            for xc in range(2):
                                nc.scalar.dma_start(
                                    out=x_sb[:, 16 * xc:16 * (xc + 1), :],
                                    in_=xT[:, 16 * xc:16 * (xc + 1),
                                           256 * tb:256 * (tb + 1)],
                                )
                            for tt in range(2):
                                ps = pps.tile([128, 512], f32, tag="ps1v")
                                nc.tensor.matmul(  # bias row outer product
                                    ps[:, :], ones_rowb[:, :], bv_sb[:, :],
                                    start=True, stop=False,
                                )
                                for h in range(32):
                                    nc.tensor.matmul(
                                        ps[:, :],
                                        x_sb[:, h, 128 * tt:128 * (tt + 1)],
                                        wv_sb[:, h, :],
                                        start=False, stop=(h == 31),
                                    )
                                nc.vector.tensor_copy(
                                    v_sb[:, 2 * tb + tt, :], ps[:, :])

                # ------------- stage 2: attention (j-outer) -------
                    if True:
                        with tc.tile_pool(name="bt2", bufs=2) as pbt, \
                             tc.tile_pool(name="p2", bufs=3) as pp, \
                             tc.tile_pool(name="bc2", bufs=1) as pbc, \
                             tc.tile_pool(name="pso", bufs=2, space="PSUM") as ppo, \
                             tc.tile_pool(name="psr", bufs=1, space="PSUM") as ppr, \
                             tc.tile_pool(name="pss", bufs=2, space="PSUM") as pp_s:
                            for j in range(HPC):
                                bt_sb = pbt.tile([128, 16, 512], bf16, tag="bt_sb")
                                nc.scalar.dma_start(
                                    out=bt_sb[:, 8:16, :], in_=btil[j, :, 8:16, :])
                                nc.scalar.dma_start(
                                    out=bt_sb[:, 0:8, :], in_=btil[j, :, 0:8, :])
                                for b in range(B):
                                    for q0 in range(4):
                                        keep = _keep_k_tiles(j, q0)
                                        # pair adjacent k-tiles: one [128,1024]
                                        # exp+mask op per pair (halves ACT/DVE
                                        # per-op overhead)
                                        pairs = [keep[i:i + 2]
                                                 for i in range(0, len(keep), 2)]
                                        out_ps = ppo.tile([128, 512], f32, tag="out_ps")
                                        rs_ps = ppr.tile([128, 512], f32, tag="rs_ps")
                                        tpos = S * b + 512 * q0
                                        LA = 2
                                        pending = []

                                        def _produce(kts, j=j, b=b, q0=q0,
                                                     bt_sb=bt_sb, tpos=tpos):
                                            n = len(kts)
                                            s_ps = pp_s.tile([128, 1024], f32, tag="s_ps")
                                            for i, kt in enumerate(kts):
                                                nc.tensor.matmul(
                                                    s_ps[:, 512 * i:512 * (i + 1)],
                                                    qk_sb[:, 4 + j,
                                                          S * b + 128 * kt:S * b + 128 * (kt + 1)],
                                                    qk_sb[:, j, tpos:tpos + 512],
                                                    start=True, stop=True,
                                                )
                                            p0 = pp.tile([128, 1024], bf16, tag="p0")
                                            nc.scalar.activation(
                                                p0[:, 0:512 * n], s_ps[:, 0:512 * n],
                                                AF.Exp, scale=SCALE
                                            )
                                            pT = pp.tile([128, 1024], bf16, tag="pT")
                                            idx0 = kts[0] - 4 * q0 + 12
                                            nc.vector.tensor_tensor(
                                                pT[:, 0:512 * n], p0[:, 0:512 * n],
                                                bt_sb[:, idx0:idx0 + n, :], op=MULT,
                                            )
                                            return pT

                                        def _consume(pi, kts, pT, j=j, b=b,
                                                     npairs_last=None,
                                                     out_ps=out_ps, rs_ps=rs_ps,
                                                     pairs=pairs):
                                            for i, kt in enumerate(kts):
                                                first = (pi == 0 and i == 0)
                                                last = (pi == len(pairs) - 1
                                                        and i == len(kts) - 1)
                                                nc.tensor.matmul(
                                                    out_ps[:, :],
                                                    v_sb[:, 16 * b + kt, 128 * j:128 * (j + 1)],
                                                    pT[:, 512 * i:512 * (i + 1)],
                                                    start=first, stop=last,
                                                )
                                                nc.tensor.matmul(
                                                    rs_ps[:, :], ones_sq_sb[:, :],
                                                    pT[:, 512 * i:512 * (i + 1)],
                                                    start=first, stop=last,
                                                )

                                        for pi, kts in enumerate(pairs):
                                            pending.append((pi, kts, _produce(kts)))
                                            if len(pending) > LA:
                                                _consume(*pending.pop(0))
                                        for item in pending:
                                            _consume(*item)
                                        rr_bc = pbc.tile([128, 512], f32, tag="rr_bc")
                                        nc.vector.reciprocal_approx_fast(
                                            out=rr_bc[:, :], in_=rs_ps[:, :])
                                        nc.vector.tensor_tensor(
                                            attnoutT[:, j, tpos:tpos + 512],
                                            out_ps[:, :], rr_bc[:, :], op=MULT,
                                        )

                    # ------------- stage 3: dense (partial, our heads) ----
                    with tc.tile_pool(name="wd3", bufs=1) as pwd, \
                         tc.tile_pool(name="o3", bufs=6) as po, \
                         tc.tile_pool(name="ps3", bufs=3, space="PSUM") as pp3:
                        wd_sb = pwd.tile([128, HPC, H], bf16)
                        nc.scalar.dma_start(out=wd_sb[:, :, :], in_=wdT[:, :, :])
                        for tt in range(T // 128):
                            for ob in range(8):
                                ps3 = pp3.tile([128, 512], f32, tag="ps3")
                                for jj in range(HPC):
                                    nc.tensor.matmul(
                                        ps3[:, :],
                                        attnoutT[:, jj, 128 * tt:128 * (tt + 1)],
                                        wd_sb[:, jj, 512 * ob:512 * (ob + 1)],
                                        start=(jj == 0), stop=(jj == HPC - 1),
                                    )
                                o_sb = po.tile([128, 512], bf16, tag="o_sb")
                                if ob % 2 == 0:
                                    nc.vector.tensor_copy(o_sb[:, :], ps3[:, :])
                                else:
                                    nc.scalar.copy(o_sb[:, :], ps3[:, :])
                                nc.sync.dma_start(
                                    out=out[128 * tt:128 * (tt + 1),
                                            512 * ob:512 * (ob + 1)],
                                    in_=o_sb[:, :],
                                )

            UNROLL = 4
            if trips == 1:
                _stages()
            elif trips % UNROLL == 0:
                if trips == UNROLL:
                    for _ in range(UNROLL):
                        _stages()
                else:
                    with tc.For_i(0, trips // UNROLL, 1):
                        for _ in range(UNROLL):
                            _stages()
            else:
                with tc.For_i(0, trips, 1):
                    _stages()

    nc.compile()
    return nc


_NC_CACHE = None


def _get_program():
    global _NC_CACHE
    if _NC_CACHE is None:
        _NC_CACHE = _build_program()
    return _NC_CACHE


def _feature_major(w_rows: np.ndarray, width: int) -> np.ndarray:
    # [width, H] weight rows -> [128, 32, width] contraction-major tiles
    return np.ascontiguousarray(
        w_rows.T.reshape(32, 128, width).transpose(1, 0, 2)
    ).astype(BF16)


def make_core_inputs(hidden_states, w_qkv, b_qkv, w_dense):
    """Shard + relayout full inputs into the 8 per-core input maps."""
    x = np.asarray(hidden_states, dtype=F32).reshape(T, H)
    xT_full = np.ascontiguousarray(
        x.T.reshape(32, 128, T).transpose(1, 0, 2)
    ).astype(BF16)
    w_qkv = np.asarray(w_qkv, dtype=F32)
    b_qkv = np.asarray(b_qkv, dtype=F32)
    w_dense = np.asarray(w_dense, dtype=F32)

    in_maps = []
    for c in range(N_CORES):
        heads = heads_for_core(c)
        rows = np.concatenate([np.arange(HD * h, HD * (h + 1)) for h in heads])
        wqk = _feature_major(
            np.concatenate([w_qkv[rows], w_qkv[H + rows]]), 1024)
        wv = _feature_major(w_qkv[2 * H + rows], 512)
        bq = b_qkv[rows].reshape(HPC, 128)
        bk = b_qkv[H + rows].reshape(HPC, 128)
        bqk_c = np.stack([*bq, *bk], axis=1).astype(F32)  # [128, 8]
        bv_c = b_qkv[2 * H + rows].reshape(1, 512).astype(BF16)

        btil_c = np.zeros((HPC, 128, 16, 512), dtype=BF16)
        kl = np.arange(128)[:, None]
        ql = np.arange(512)[None, :]
        for j, h in enumerate(heads):
            slope = _SLOPES[h]
            for idx in range(16):
                rel = (128 * (idx - 12) + kl - ql).astype(np.float64)
                tilev = np.where(rel <= 0, np.exp(slope * rel), 0.0)
                btil_c[j, :, idx, :] = tilev.astype(BF16)

        wd_c = np.stack(
            [np.ascontiguousarray(w_dense[:, HD * h:HD * (h + 1)].T) for h in heads],
            axis=1,
        ).astype(BF16)  # [128, HPC, H]

        in_maps.append({
            "ones_rb": np.ones((1, 128), dtype=BF16),
            "ones_sq": np.ones((128, 128), dtype=BF16),
            "xT": xT_full,
            "wqkT": wqk, "wvT": wv,
            "bqk": bqk_c, "bv": bv_c,
            "btil": btil_c, "wdT": wd_c,
        })
    return in_maps


def kernel(hidden_states, w_qkv, b_qkv, w_dense, b_dense):
    from concourse.bass_utils import run_bass_kernel_spmd

    nc = _get_program()
    in_maps = make_core_inputs(hidden_states, w_qkv, b_qkv, w_dense)
    res = run_bass_kernel_spmd(nc, in_maps, core_ids=list(range(N_CORES)))
    acc = np.zeros((T, H), dtype=np.float32)
    for c in range(N_CORES):
        acc += res.results[c]["out"]
    acc += np.asarray(b_dense, dtype=np.float32)[None, :]
    return acc.reshape(B, S, H).astype(np.float32)
